# revision 11
# baseline (speedup 1.0000x reference)
"""L2-bounded LTI cell (SSM scan) as a truncated convolution on TRN2.

Math: the reference computes, per batch b:
    x_{t+1} = x_t @ A.T + u_t @ B.T
    y_t     = x_t @ C.T + u_t @ D.T
with outputs x_seq[t] = x_t (pre-update state, x_0 = x0) and y_seq[t] = y_t.

K = K_raw / (||K_raw||_2 + 0.002) is a strict contraction and A is similar
to a submatrix of K, so ||A^m||_2 decays geometrically (measured:
||A^20|| ~ 3.6e-7, ||A^24|| ~ 6e-9). Hence

    x_t = x0 @ At^t + sum_{m=0}^{t-1} u_{t-1-m} @ G_m,   G_m = Bt @ At^m

truncated at m < M_TAPS has error far below fp32 roundoff. This turns the
sequential scan into a causal convolution: M_TAPS accumulating 128x128x512
matmuls per output tile, with the rhs being shifted windows of a
zero-padded, transposed u buffer resident in SBUF.

Precision (validated against the reference in simulation):
 - taps 0..K_SPLIT-1 carry most of the signal -> 3-pass bf16 split
   (Gh*uh + Gh*ul + Gl*uh with X = Xh + Xl bf16 hi/lo decomposition),
   which is fp32-class accurate and runs at full PE rate.
 - taps K_SPLIT.. run as single float32r matmuls (TRN2 "round" fp32 mode,
   ~12-bit mantissa, full PE rate at free dim >= 256).
 - y = x @ Ct + u @ Dt uses 3-pass bf16 for both terms (y scale is ~30x
   smaller than x scale, so single bf16/fp32r is not enough there).
Measured end-to-end accuracy of this scheme vs the fp32 reference:
x ~ 1e-5, y ~ 8e-5 absmax-relative (fp32 noise floor is ~6e-6/9e-6).

Sharding: batch 32 -> 4 per core, 8 cores, SPMD, no collectives.
Layout: on-chip everything is (d=128 partitions) x (time free dim); the
host pre-transposes u and post-transposes y/x (host work, not HW time).
The tiny x0 @ At^t boundary term (same geometric decay) is added on host.

Every PSUM accumulation group starts with a bf16 matmul: bf16 weights use
a separate LDWEIGHTS instruction so multi-sem waits can be legalized,
while fp32/fp32r self-loading matmuls only support a single wait slot.
"""

import os
from functools import lru_cache

import numpy as np

B_FULL, T, D = 32, 4096, 128
N_CORES = 8
B_LOCAL = B_FULL // N_CORES  # 4

M_TAPS = int(os.environ.get("LTI_M", "20"))  # conv taps
K_SPLIT = int(os.environ.get("LTI_KSPLIT", "4"))  # 3-pass bf16 taps
M_X0 = 64  # host-side x0-term horizon; ||A^64|| ~ 3e-26
N_TILE = 512  # matmul free dim (one fp32 PSUM bank)

_last_result = None  # BassKernelResults of the most recent run (for test.py)


def _host_matrices(S, K_raw):
    """Mirror reference._ssm_matrices bit-for-bit: fp32 jax on CPU."""
    import jax
    import jax.numpy as jnp

    cpu = jax.devices("cpu")[0]
    with jax.default_device(cpu):
        d_x = S.shape[0]
        sigma = jnp.maximum(jnp.linalg.norm(jnp.asarray(K_raw), ord=2), 1e-5)
        K = jnp.asarray(K_raw) / (sigma + 0.002)
        K11 = K[:d_x, :d_x]
        K12 = K[:d_x, d_x:]
        K21 = K[d_x:, :d_x]
        K22 = K[d_x:, d_x:]
        Sinv = jnp.linalg.inv(jnp.asarray(S))
        A = Sinv @ K11 @ jnp.asarray(S)
        Bm = Sinv @ K12  # GAMMA = 1.0
        C = K21 @ jnp.asarray(S)
        Dm = K22
        return (np.asarray(A), np.asarray(Bm), np.asarray(C), np.asarray(Dm))


@lru_cache(maxsize=2)
def _build(m_taps: int, k_split: int):
    import concourse.mybir as mybir
    import concourse.tile as tile
    from concourse import bacc

    F32 = mybir.dt.float32
    F32R = mybir.dt.float32r
    BF16 = mybir.dt.bfloat16
    tp = T + m_taps
    n_tiles = T // N_TILE
    n_tail = m_taps - k_split

    nc = bacc.Bacc("TRN2", target_bir_lowering=False, num_devices=N_CORES)
    u_d = nc.dram_tensor("u", [B_LOCAL, D, tp], F32, kind="ExternalInput")
    # fp32r copy of u for the tail taps: fp32r matmul operands must come
    # from fp32r tensors (BIR verifier), and the bf16 hi/lo split needs the
    # unrounded fp32 u, so the buffer is loaded twice under the two dtypes.
    ur_d = nc.dram_tensor("ur", [B_LOCAL, D, tp], F32R, kind="ExternalInput")
    gs_d = nc.dram_tensor("gs", [D, 2 * k_split, D], BF16, kind="ExternalInput")
    gr_d = nc.dram_tensor("gr", [D, n_tail, D], F32R, kind="ExternalInput")
    cd_d = nc.dram_tensor("cd", [D, 6, D], BF16, kind="ExternalInput")
    y_d = nc.dram_tensor("y", [B_LOCAL, D, T], F32, kind="ExternalOutput")
    x_d = nc.dram_tensor("x", [B_LOCAL, D, T], F32, kind="ExternalOutput")

    with tile.TileContext(nc) as tc:
        with (
            tc.tile_pool(name="const", bufs=1) as const,
            tc.tile_pool(name="upool", bufs=2) as upool,
            tc.tile_pool(name="urpool", bufs=2) as urpool,
            tc.tile_pool(name="uhpool", bufs=2) as uhpool,
            tc.tile_pool(name="ulpool", bufs=2) as ulpool,
            tc.tile_pool(name="xf", bufs=3) as xf_pool,
            tc.tile_pool(name="xh", bufs=3) as xh_pool,
            tc.tile_pool(name="xl", bufs=3) as xl_pool,
            tc.tile_pool(name="yf", bufs=3) as yf_pool,
            tc.tile_pool(name="px", bufs=3, space="PSUM") as px_pool,
            tc.tile_pool(name="py", bufs=3, space="PSUM") as py_pool,
        ):
            gs_sb = const.tile([D, 2 * k_split, D], BF16)
            nc.sync.dma_start(gs_sb[:], gs_d[:])
            gr_sb = const.tile([D, n_tail, D], F32R)
            nc.sync.dma_start(gr_sb[:], gr_d[:])
            cd_sb = const.tile([D, 6, D], BF16)
            nc.sync.dma_start(cd_sb[:], cd_d[:])

            for b in range(B_LOCAL):
                u_sb = upool.tile([D, tp], F32)
                nc.sync.dma_start(u_sb[:], u_d[b])
                ur_sb = urpool.tile([D, tp], F32R)
                nc.sync.dma_start(ur_sb[:], ur_d[b])
                uh_sb = uhpool.tile([D, tp], BF16)
                nc.vector.tensor_copy(uh_sb[:], u_sb[:])
                ul_sb = ulpool.tile([D, tp], BF16)
                nc.vector.tensor_sub(ul_sb[:], u_sb[:], uh_sb[:])

                for j in range(n_tiles):
                    t0 = j * N_TILE
                    px = px_pool.tile([D, N_TILE], F32)
                    n_mm = 3 * k_split + n_tail
                    k = 0
                    for m in range(k_split):
                        s = m_taps + t0 - 1 - m
                        gh = gs_sb[:, 2 * m, :]
                        gl = gs_sb[:, 2 * m + 1, :]
                        for lhsT, rhs in (
                            (gh, uh_sb[:, s : s + N_TILE]),
                            (gh, ul_sb[:, s : s + N_TILE]),
                            (gl, uh_sb[:, s : s + N_TILE]),
                        ):
                            nc.tensor.matmul(
                                px[:], lhsT, rhs,
                                start=(k == 0), stop=(k == n_mm - 1),
                            )
                            k += 1
                    for m in range(k_split, m_taps):
                        s = m_taps + t0 - 1 - m
                        nc.tensor.matmul(
                            px[:],
                            gr_sb[:, m - k_split, :],
                            ur_sb[:, s : s + N_TILE],
                            start=(k == 0), stop=(k == n_mm - 1),
                        )
                        k += 1

                    xf = xf_pool.tile([D, N_TILE], F32)
                    nc.vector.tensor_copy(xf[:], px[:])
                    xh = xh_pool.tile([D, N_TILE], BF16)
                    nc.vector.tensor_copy(xh[:], px[:])
                    xl = xl_pool.tile([D, N_TILE], BF16)
                    nc.vector.tensor_sub(xl[:], px[:], xh[:])

                    py = py_pool.tile([D, N_TILE], F32)
                    s0 = m_taps + t0
                    uhw = uh_sb[:, s0 : s0 + N_TILE]
                    ulw = ul_sb[:, s0 : s0 + N_TILE]
                    y_parts = (
                        (cd_sb[:, 0, :], xh[:]),  # Cth * xh
                        (cd_sb[:, 0, :], xl[:]),  # Cth * xl
                        (cd_sb[:, 1, :], xh[:]),  # Ctl * xh
                        (cd_sb[:, 2, :], uhw),    # Dth * uh
                        (cd_sb[:, 2, :], ulw),    # Dth * ul
                        (cd_sb[:, 3, :], uhw),    # Dtl * uh
                    )
                    for i, (lhsT, rhs) in enumerate(y_parts):
                        nc.tensor.matmul(
                            py[:], lhsT, rhs,
                            start=(i == 0), stop=(i == len(y_parts) - 1),
                        )
                    yf = yf_pool.tile([D, N_TILE], F32)
                    nc.vector.tensor_copy(yf[:], py[:])

                    nc.sync.dma_start(x_d[b][:, t0 : t0 + N_TILE], xf[:])
                    nc.sync.dma_start(y_d[b][:, t0 : t0 + N_TILE], yf[:])
    nc.compile()
    return nc


def _pack_inputs(u, x0, S, K_raw, m, ks):
    import ml_dtypes

    bf = ml_dtypes.bfloat16
    A, Bm, C, Dm = _host_matrices(S, K_raw)

    At = A.T.astype(np.float64)
    G = np.empty((m, D, D), dtype=np.float64)
    G[0] = Bm.T.astype(np.float64)
    for i in range(1, m):
        G[i] = G[i - 1] @ At

    # Split taps: interleaved (Gh, Gl) pairs, packed [d_in, 2*ks, d_state].
    gs = np.empty((ks, 2, D, D), dtype=np.float32)
    for i in range(ks):
        g32 = G[i].astype(np.float32)
        gh = g32.astype(bf).astype(np.float32)
        gs[i, 0] = gh
        gs[i, 1] = g32 - gh
    gs_host = np.ascontiguousarray(
        gs.reshape(2 * ks, D, D).transpose(1, 0, 2)
    ).astype(bf)

    gr_host = np.ascontiguousarray(
        G[ks:].astype(np.float32).transpose(1, 0, 2)
    )

    # cd: slots (Cth, Ctl, Dth, Dtl, 0, 0) packed [d, 6, d].
    cd = np.zeros((6, D, D), dtype=np.float32)
    Ct = C.T.astype(np.float32)
    Dt = Dm.T.astype(np.float32)
    cd[0] = Ct.astype(bf).astype(np.float32)
    cd[1] = Ct - cd[0]
    cd[2] = Dt.astype(bf).astype(np.float32)
    cd[3] = Dt - cd[2]
    cd_host = np.ascontiguousarray(cd.transpose(1, 0, 2)).astype(bf)

    in_maps = []
    for c in range(N_CORES):
        up = np.zeros((B_LOCAL, D, T + m), dtype=np.float32)
        for b in range(B_LOCAL):
            up[b, :, m:] = u[c * B_LOCAL + b].T
        in_maps.append(
            {"u": up, "ur": up, "gs": gs_host, "gr": gr_host, "cd": cd_host}
        )
    return in_maps, A, C


def kernel(u, x0, S, K_raw):
    global _last_result
    from concourse.bass_utils import run_bass_kernel_spmd

    m, ks = M_TAPS, K_SPLIT
    u = np.asarray(u, dtype=np.float32)
    x0 = np.asarray(x0, dtype=np.float32)
    S = np.asarray(S, dtype=np.float32)
    K_raw = np.asarray(K_raw, dtype=np.float32)

    in_maps, A, C = _pack_inputs(u, x0, S, K_raw, m, ks)
    nc = _build(m, ks)
    res = run_bass_kernel_spmd(nc, in_maps, core_ids=list(range(N_CORES)))
    _last_result = res

    y_seq = np.empty((B_FULL, T, D), dtype=np.float32)
    x_seq = np.empty((B_FULL, T, D), dtype=np.float32)
    for c in range(N_CORES):
        ry, rx = res.results[c]["y"], res.results[c]["x"]
        for b in range(B_LOCAL):
            y_seq[c * B_LOCAL + b] = ry[b].T
            x_seq[c * B_LOCAL + b] = rx[b].T

    # x0 boundary term: x_t += x0 @ At^t, y_t += (x0 @ At^t) @ Ct, t < M_X0.
    At = A.T.astype(np.float64)
    Ct64 = C.T.astype(np.float64)
    xc = x0.astype(np.float64)
    for t in range(M_X0):
        x_seq[:, t, :] += xc.astype(np.float32)
        y_seq[:, t, :] += (xc @ Ct64).astype(np.float32)
        xc = xc @ At

    return (y_seq, x_seq)


# revision 12
# speedup vs baseline: 2.0481x; 2.0481x over previous
"""L2-bounded LTI cell (SSM scan) as a truncated convolution on TRN2.

Math: the reference computes, per batch b:
    x_{t+1} = x_t @ A.T + u_t @ B.T
    y_t     = x_t @ C.T + u_t @ D.T
with outputs x_seq[t] = x_t (pre-update state, x_0 = x0) and y_seq[t] = y_t.

K = K_raw / (||K_raw||_2 + 0.002) is a strict contraction and A is similar
to a submatrix of K, so ||A^m||_2 decays geometrically (measured:
||A^20|| ~ 3.6e-7, ||A^24|| ~ 6e-9). Hence

    x_t = x0 @ At^t + sum_{m=0}^{t-1} u_{t-1-m} @ G_m,   G_m = Bt @ At^m

truncated at m < M_TAPS has error far below fp32 roundoff. This turns the
sequential scan into a causal convolution: M_TAPS accumulating 128x128x512
matmuls per output tile, with the rhs being shifted windows of a
zero-padded, transposed u buffer resident in SBUF.

Precision (validated against the reference in simulation):
 - taps 0..K_SPLIT-1 carry most of the signal -> 3-pass bf16 split
   (Gh*uh + Gh*ul + Gl*uh with X = Xh + Xl bf16 hi/lo decomposition),
   which is fp32-class accurate and runs at full PE rate.
 - taps K_SPLIT.. run as single float32r matmuls (TRN2 "round" fp32 mode,
   ~12-bit mantissa, full PE rate at free dim >= 256).
 - y = x @ Ct + u @ Dt uses 3-pass bf16 for both terms (y scale is ~30x
   smaller than x scale, so single bf16/fp32r is not enough there).
Measured end-to-end accuracy of this scheme vs the fp32 reference:
x ~ 1e-5, y ~ 8e-5 absmax-relative (fp32 noise floor is ~6e-6/9e-6).

Sharding: batch 32 -> 4 per core, 8 cores, SPMD, no collectives.
Layout: on-chip everything is (d=128 partitions) x (time free dim); the
host pre-transposes u and post-transposes y/x (host work, not HW time).
The tiny x0 @ At^t boundary term (same geometric decay) is added on host.

Every PSUM accumulation group starts with a bf16 matmul: bf16 weights use
a separate LDWEIGHTS instruction so multi-sem waits can be legalized,
while fp32/fp32r self-loading matmuls only support a single wait slot.
"""

import os
from functools import lru_cache

import numpy as np

B_FULL, T, D = 32, 4096, 128
N_CORES = 8
B_LOCAL = B_FULL // N_CORES  # 4

M_TAPS = int(os.environ.get("LTI_M", "12"))  # conv taps
K_SPLIT = int(os.environ.get("LTI_KSPLIT", "3"))  # 3-pass bf16 taps
M_X0 = 64  # host-side x0-term horizon; ||A^64|| ~ 3e-26
N_TILE = 512  # matmul free dim (one fp32 PSUM bank)

_last_result = None  # BassKernelResults of the most recent run (for test.py)


def _host_matrices(S, K_raw):
    """Mirror reference._ssm_matrices bit-for-bit: fp32 jax on CPU."""
    import jax
    import jax.numpy as jnp

    cpu = jax.devices("cpu")[0]
    with jax.default_device(cpu):
        d_x = S.shape[0]
        sigma = jnp.maximum(jnp.linalg.norm(jnp.asarray(K_raw), ord=2), 1e-5)
        K = jnp.asarray(K_raw) / (sigma + 0.002)
        K11 = K[:d_x, :d_x]
        K12 = K[:d_x, d_x:]
        K21 = K[d_x:, :d_x]
        K22 = K[d_x:, d_x:]
        Sinv = jnp.linalg.inv(jnp.asarray(S))
        A = Sinv @ K11 @ jnp.asarray(S)
        Bm = Sinv @ K12  # GAMMA = 1.0
        C = K21 @ jnp.asarray(S)
        Dm = K22
        return (np.asarray(A), np.asarray(Bm), np.asarray(C), np.asarray(Dm))


@lru_cache(maxsize=2)
def _build(m_taps: int, k_split: int):
    import concourse.mybir as mybir
    import concourse.tile as tile
    from concourse import bacc

    F32 = mybir.dt.float32
    F32R = mybir.dt.float32r
    BF16 = mybir.dt.bfloat16
    tp = T + m_taps
    n_tiles = T // N_TILE
    n_tail = m_taps - k_split

    nc = bacc.Bacc("TRN2", target_bir_lowering=False, num_devices=N_CORES)
    u_d = nc.dram_tensor("u", [B_LOCAL, D, tp], F32, kind="ExternalInput")
    # fp32r copy of u for the tail taps: fp32r matmul operands must come
    # from fp32r tensors (BIR verifier), and the bf16 hi/lo split needs the
    # unrounded fp32 u, so the buffer is loaded twice under the two dtypes.
    ur_d = nc.dram_tensor("ur", [B_LOCAL, D, tp], F32R, kind="ExternalInput")
    gs_d = nc.dram_tensor("gs", [D, 2 * k_split, D], BF16, kind="ExternalInput")
    gr_d = nc.dram_tensor("gr", [D, n_tail, D], F32R, kind="ExternalInput")
    cd_d = nc.dram_tensor("cd", [D, 6, D], BF16, kind="ExternalInput")
    y_d = nc.dram_tensor("y", [B_LOCAL, D, T], F32, kind="ExternalOutput")
    x_d = nc.dram_tensor("x", [B_LOCAL, D, T], F32, kind="ExternalOutput")

    with tile.TileContext(nc) as tc:
        with (
            tc.tile_pool(name="const", bufs=1) as const,
            tc.tile_pool(name="upool", bufs=2) as upool,
            tc.tile_pool(name="urpool", bufs=2) as urpool,
            tc.tile_pool(name="uhpool", bufs=2) as uhpool,
            tc.tile_pool(name="ulpool", bufs=2) as ulpool,
            tc.tile_pool(name="xf", bufs=3) as xf_pool,
            tc.tile_pool(name="xh", bufs=3) as xh_pool,
            tc.tile_pool(name="xl", bufs=3) as xl_pool,
            tc.tile_pool(name="yf", bufs=3) as yf_pool,
            tc.tile_pool(name="px", bufs=3, space="PSUM") as px_pool,
            tc.tile_pool(name="py", bufs=3, space="PSUM") as py_pool,
        ):
            gs_sb = const.tile([D, 2 * k_split, D], BF16)
            nc.sync.dma_start(gs_sb[:], gs_d[:])
            gr_sb = const.tile([D, n_tail, D], F32R)
            nc.sync.dma_start(gr_sb[:], gr_d[:])
            cd_sb = const.tile([D, 6, D], BF16)
            nc.sync.dma_start(cd_sb[:], cd_d[:])

            for b in range(B_LOCAL):
                u_sb = upool.tile([D, tp], F32)
                nc.sync.dma_start(u_sb[:], u_d[b])
                ur_sb = urpool.tile([D, tp], F32R)
                nc.sync.dma_start(ur_sb[:], ur_d[b])
                uh_sb = uhpool.tile([D, tp], BF16)
                nc.vector.tensor_copy(uh_sb[:], u_sb[:])
                ul_sb = ulpool.tile([D, tp], BF16)
                nc.vector.tensor_sub(ul_sb[:], u_sb[:], uh_sb[:])

                for j in range(n_tiles):
                    t0 = j * N_TILE
                    px = px_pool.tile([D, N_TILE], F32)
                    n_mm = 3 * k_split + n_tail
                    k = 0
                    for m in range(k_split):
                        s = m_taps + t0 - 1 - m
                        gh = gs_sb[:, 2 * m, :]
                        gl = gs_sb[:, 2 * m + 1, :]
                        for lhsT, rhs in (
                            (gh, uh_sb[:, s : s + N_TILE]),
                            (gh, ul_sb[:, s : s + N_TILE]),
                            (gl, uh_sb[:, s : s + N_TILE]),
                        ):
                            nc.tensor.matmul(
                                px[:], lhsT, rhs,
                                start=(k == 0), stop=(k == n_mm - 1),
                            )
                            k += 1
                    for m in range(k_split, m_taps):
                        s = m_taps + t0 - 1 - m
                        nc.tensor.matmul(
                            px[:],
                            gr_sb[:, m - k_split, :],
                            ur_sb[:, s : s + N_TILE],
                            start=(k == 0), stop=(k == n_mm - 1),
                        )
                        k += 1

                    xf = xf_pool.tile([D, N_TILE], F32)
                    nc.vector.tensor_copy(xf[:], px[:])
                    xh = xh_pool.tile([D, N_TILE], BF16)
                    nc.vector.tensor_copy(xh[:], px[:])
                    xl = xl_pool.tile([D, N_TILE], BF16)
                    nc.vector.tensor_sub(xl[:], px[:], xh[:])

                    py = py_pool.tile([D, N_TILE], F32)
                    s0 = m_taps + t0
                    uhw = uh_sb[:, s0 : s0 + N_TILE]
                    ulw = ul_sb[:, s0 : s0 + N_TILE]
                    y_parts = (
                        (cd_sb[:, 0, :], xh[:]),  # Cth * xh
                        (cd_sb[:, 0, :], xl[:]),  # Cth * xl
                        (cd_sb[:, 1, :], xh[:]),  # Ctl * xh
                        (cd_sb[:, 2, :], uhw),    # Dth * uh
                        (cd_sb[:, 2, :], ulw),    # Dth * ul
                        (cd_sb[:, 3, :], uhw),    # Dtl * uh
                    )
                    for i, (lhsT, rhs) in enumerate(y_parts):
                        nc.tensor.matmul(
                            py[:], lhsT, rhs,
                            start=(i == 0), stop=(i == len(y_parts) - 1),
                        )
                    yf = yf_pool.tile([D, N_TILE], F32)
                    nc.vector.tensor_copy(yf[:], py[:])

                    nc.sync.dma_start(x_d[b][:, t0 : t0 + N_TILE], xf[:])
                    nc.sync.dma_start(y_d[b][:, t0 : t0 + N_TILE], yf[:])
    nc.compile()
    return nc


def _pack_inputs(u, x0, S, K_raw, m, ks):
    import ml_dtypes

    bf = ml_dtypes.bfloat16
    A, Bm, C, Dm = _host_matrices(S, K_raw)

    At = A.T.astype(np.float64)
    G = np.empty((m, D, D), dtype=np.float64)
    G[0] = Bm.T.astype(np.float64)
    for i in range(1, m):
        G[i] = G[i - 1] @ At

    # Split taps: interleaved (Gh, Gl) pairs, packed [d_in, 2*ks, d_state].
    gs = np.empty((ks, 2, D, D), dtype=np.float32)
    for i in range(ks):
        g32 = G[i].astype(np.float32)
        gh = g32.astype(bf).astype(np.float32)
        gs[i, 0] = gh
        gs[i, 1] = g32 - gh
    gs_host = np.ascontiguousarray(
        gs.reshape(2 * ks, D, D).transpose(1, 0, 2)
    ).astype(bf)

    gr_host = np.ascontiguousarray(
        G[ks:].astype(np.float32).transpose(1, 0, 2)
    )

    # cd: slots (Cth, Ctl, Dth, Dtl, 0, 0) packed [d, 6, d].
    cd = np.zeros((6, D, D), dtype=np.float32)
    Ct = C.T.astype(np.float32)
    Dt = Dm.T.astype(np.float32)
    cd[0] = Ct.astype(bf).astype(np.float32)
    cd[1] = Ct - cd[0]
    cd[2] = Dt.astype(bf).astype(np.float32)
    cd[3] = Dt - cd[2]
    cd_host = np.ascontiguousarray(cd.transpose(1, 0, 2)).astype(bf)

    in_maps = []
    for c in range(N_CORES):
        up = np.zeros((B_LOCAL, D, T + m), dtype=np.float32)
        for b in range(B_LOCAL):
            up[b, :, m:] = u[c * B_LOCAL + b].T
        in_maps.append(
            {"u": up, "ur": up, "gs": gs_host, "gr": gr_host, "cd": cd_host}
        )
    return in_maps, A, C


def kernel(u, x0, S, K_raw):
    global _last_result
    from concourse.bass_utils import run_bass_kernel_spmd

    m, ks = M_TAPS, K_SPLIT
    u = np.asarray(u, dtype=np.float32)
    x0 = np.asarray(x0, dtype=np.float32)
    S = np.asarray(S, dtype=np.float32)
    K_raw = np.asarray(K_raw, dtype=np.float32)

    in_maps, A, C = _pack_inputs(u, x0, S, K_raw, m, ks)
    nc = _build(m, ks)
    res = run_bass_kernel_spmd(nc, in_maps, core_ids=list(range(N_CORES)))
    _last_result = res

    y_seq = np.empty((B_FULL, T, D), dtype=np.float32)
    x_seq = np.empty((B_FULL, T, D), dtype=np.float32)
    for c in range(N_CORES):
        ry, rx = res.results[c]["y"], res.results[c]["x"]
        for b in range(B_LOCAL):
            y_seq[c * B_LOCAL + b] = ry[b].T
            x_seq[c * B_LOCAL + b] = rx[b].T

    # x0 boundary term: x_t += x0 @ At^t, y_t += (x0 @ At^t) @ Ct, t < M_X0.
    At = A.T.astype(np.float64)
    Ct64 = C.T.astype(np.float64)
    xc = x0.astype(np.float64)
    for t in range(M_X0):
        x_seq[:, t, :] += xc.astype(np.float32)
        y_seq[:, t, :] += (xc @ Ct64).astype(np.float32)
        xc = xc @ At

    return (y_seq, x_seq)
